# revision 10
# baseline (speedup 1.0000x reference)
"""CrossLayerAttention Trainium2 kernel.

Sharding: 8 cores = 4 batches x 2 head-groups (8 heads each).
Each core computes, for its (batch b, head-group g):
  qT[j, t] = sum_h wqT[h, j] * hsT[h, t]          (q projection, transposed)
  RoPE on qT (d on partitions), scale 1/sqrt(hd) folded into cos/sin tables
  scoresT[s, t] = sum_d k[d, s] * qT[d, t] + maskT[s, t]
  probsT = exp(scoresT)            (no max-sub: scores are O(1) by construction)
  rowsum[t] = sum_s probsT[s, t]   (ones-vector matmul)
  attnT[d, t] = (sum_s v[s, d] * probsT[s, t]) / rowsum[t]
  outT[j, t] = sum_c woT[c, j] * attnT[c, t]      (partial over this head group)
Host sums the two head-group partials per batch, transposes back, adds bo.

All matmuls run as float32r (single-pass fp32, ~1e-4 rel err).
"""

import sys

sys.path.insert(0, "/opt/trn_rl_repo")
sys.path.insert(0, "/root/.axon_site/_ro/trn_rl_repo")

from contextlib import ExitStack

import numpy as np

import concourse.bass as bass
import concourse.tile as tile
from concourse import mybir
from concourse.bass_utils import run_bass_kernel_spmd

B, S, H, NH = 4, 1024, 2048, 16
HD = H // NH  # 128
P = 128
NHG = NH // 2  # heads per core = 8
JW = NHG * HD  # local j width = 1024
f32 = mybir.dt.float32
f32r = mybir.dt.float32r
TH = 2  # t halves of 512
FD = 512


def _split_multiwaits(nc):
    """Walrus only supports one sync-wait slot per 64B instruction (and the
    fused fp32r weight-load takes exactly one). Hoist extra waits onto NoOps."""
    n = 0
    for f in nc.m.functions:
        for blk in f.blocks:
            new = []
            for inst in blk.instructions:
                si = inst.sync_info
                waits = list(si.on_wait) if si and si.on_wait else []
                if len(waits) > 1:
                    for w in waits[:-1]:
                        nop = mybir.InstNoOp(name=f"I-waitsplit-{n}")
                        n += 1
                        nop.engine = inst.engine
                        nop.sync_info = mybir.SyncInfo(on_wait=[w], on_update=[])
                        new.append(nop)
                    inst.sync_info = mybir.SyncInfo(
                        on_wait=[waits[-1]], on_update=list(si.on_update or [])
                    )
                new.append(inst)
            blk.instructions = new
    return n


def _build_program():
    nc = bass.Bass("TRN2", target_bir_lowering=False, debug=False)
    hsT = nc.dram_tensor("hsT", [H, S], f32r, kind="ExternalInput").ap()
    wqT = nc.dram_tensor("wqT", [H, JW], f32r, kind="ExternalInput").ap()
    bqT = nc.dram_tensor("bqT", [P, NHG], f32, kind="ExternalInput").ap()
    kk = nc.dram_tensor("kk", [NHG, HD, S], f32r, kind="ExternalInput").ap()
    vv = nc.dram_tensor("vv", [NHG, S, HD], f32r, kind="ExternalInput").ap()
    maskT = nc.dram_tensor("maskT", [S, S], f32, kind="ExternalInput").ap()
    cosT = nc.dram_tensor("cosT", [HD, S], f32, kind="ExternalInput").ap()
    sinT = nc.dram_tensor("sinT", [HD, S], f32, kind="ExternalInput").ap()
    woT = nc.dram_tensor("woT", [JW, H], f32r, kind="ExternalInput").ap()
    ones_d = nc.dram_tensor("ones_d", [P, P], f32r, kind="ExternalInput").ap()
    outT = nc.dram_tensor("outT", [H, S], f32, kind="ExternalOutput").ap()

    AF = mybir.ActivationFunctionType
    ALU = mybir.AluOpType
    HC = H // P  # 16 h-chunks

    with (
        tile.TileContext(nc) as tc,
        nc.allow_low_precision("float32r is fp32-width; rounding intended for fast matmul"),
        ExitStack() as ctx,
    ):
        # ---- persistent pools (whole kernel) ----
        const_pool = ctx.enter_context(tc.tile_pool(name="const", bufs=1))
        qTr_pool = ctx.enter_context(tc.tile_pool(name="qTr", bufs=1))
        attn_pool = ctx.enter_context(tc.tile_pool(name="attn", bufs=1))
        mask_pool = ctx.enter_context(tc.tile_pool(name="mask", bufs=1))

        ones_sb = const_pool.tile([P, 1], f32r, tag="ones", name="ones")
        nc.sync.dma_start(ones_sb[:], ones_d[:, 0:1])
        ones_row = const_pool.tile([1, P], f32r, tag="ones_row", name="ones_row")
        nc.sync.dma_start(ones_row[:], ones_d[0:1, :])
        cos_sb = const_pool.tile([P, S], f32, tag="cos", name="cos")
        nc.sync.dma_start(cos_sb[:], cosT)
        sin_sb = const_pool.tile([P, S], f32, tag="sin", name="sin")
        nc.sync.dma_start(sin_sb[:], sinT)
        bq_sb = const_pool.tile([P, NHG], f32, tag="bq", name="bq")
        nc.sync.dma_start(bq_sb[:], bqT)

        mask_sb = [mask_pool.tile([P, S], f32, tag=f"mask{st}", name=f"mask{st}") for st in range(8)]
        for st in range(8):
            nc.sync.dma_start(mask_sb[st][:], maskT[st * P : (st + 1) * P, :])

        qTr = [qTr_pool.tile([P, S], f32r, tag=f"qTr{h}", name=f"qTr{h}") for h in range(NHG)]
        attn_sb = [attn_pool.tile([P, S], f32r, tag=f"attn{h}", name=f"attn{h}") for h in range(NHG)]

        # ---- phase 1: q projection + RoPE ----
        with ExitStack() as p1:
            hs_pool = p1.enter_context(tc.tile_pool(name="hs", bufs=1))
            wq_pool = p1.enter_context(tc.tile_pool(name="wq", bufs=2))
            rope_pool = p1.enter_context(tc.tile_pool(name="rope", bufs=2))
            qps_pool = p1.enter_context(tc.tile_pool(name="qps", bufs=4, space="PSUM"))

            hs_sb = [hs_pool.tile([P, S], f32r, tag=f"hs{hc}", name=f"hs{hc}") for hc in range(HC)]
            for hc in range(HC):
                nc.sync.dma_start(hs_sb[hc][:], hsT[hc * P : (hc + 1) * P, :])

            wqT_r = wqT.rearrange("(hc p) j -> p hc j", p=P)
            for h in range(NHG):
                wq_sb = wq_pool.tile([P, HC, HD], f32r, tag="wq", name="wq")
                nc.sync.dma_start(wq_sb[:], wqT_r[:, :, h * HD : (h + 1) * HD])
                for th in range(TH):
                    ts = slice(th * FD, (th + 1) * FD)
                    qps = qps_pool.tile([P, FD], f32, tag="qps", name="qps")
                    for hc in range(HC):
                        nc.tensor.matmul(
                            qps[:],
                            wq_sb[:, hc, :],
                            hs_sb[hc][:, ts],
                            start=(hc == 0),
                            stop=(hc == HC - 1),
                        )
                    qraw = rope_pool.tile([P, FD], f32, tag="qraw", name="qraw")
                    nc.scalar.activation(
                        qraw[:], qps[:], AF.Identity, bias=bq_sb[:, h : h + 1], scale=1.0
                    )
                    qsw = rope_pool.tile([P, FD], f32, tag="qsw", name="qsw")
                    nc.sync.dma_start(qsw[0:64, :], qraw[64:128, :])
                    nc.sync.dma_start(qsw[64:128, :], qraw[0:64, :])
                    nc.vector.tensor_tensor(qraw[:], qraw[:], cos_sb[:, ts], ALU.mult)
                    nc.vector.tensor_tensor(qsw[:], qsw[:], sin_sb[:, ts], ALU.mult)
                    nc.vector.tensor_tensor(qTr[h][:, ts], qraw[:], qsw[:], ALU.add)

        # ---- phase 2: attention per head ----
        with ExitStack() as p2:
            kv_pool = p2.enter_context(tc.tile_pool(name="kv", bufs=2))
            pr_pool = p2.enter_context(tc.tile_pool(name="pr", bufs=3))
            nrm_pool = p2.enter_context(tc.tile_pool(name="nrm", bufs=2))
            sc_pool = p2.enter_context(tc.tile_pool(name="scps", bufs=2, space="PSUM"))
            sm_pool = p2.enter_context(tc.tile_pool(name="smps", bufs=1, space="PSUM"))
            av_pool = p2.enter_context(tc.tile_pool(name="avps", bufs=1, space="PSUM"))
            bc_pool = p2.enter_context(tc.tile_pool(name="bcps", bufs=2, space="PSUM"))

            for h in range(NHG):
                k_sb = kv_pool.tile([P, S], f32r, tag="k", name="k")
                nc.sync.dma_start(k_sb[:], kk[h, :, :])
                v_sb = kv_pool.tile([P, 8, HD], f32r, tag="v", name="v")
                nc.sync.dma_start(v_sb[:], vv[h, :, :].rearrange("(st p) d -> p st d", p=P))

                attn_ps = av_pool.tile([P, S], f32, tag="avps", name="avps")
                sum_ps = [sm_pool.tile([1, FD], f32, tag=f"smps{th}", name=f"smps{th}") for th in range(TH)]
                for th in range(TH):
                    ts = slice(th * FD, (th + 1) * FD)
                    for st in range(8):
                        scps = sc_pool.tile([P, FD], f32, tag="scps", name="scps")
                        nc.tensor.matmul(
                            scps[:],
                            k_sb[:, st * P : (st + 1) * P],
                            qTr[h][:, ts],
                            start=True,
                            stop=True,
                        )
                        nc.vector.tensor_tensor(scps[:], scps[:], mask_sb[st][:, ts], ALU.add)
                        probs = pr_pool.tile([P, FD], f32r, tag="probs", name="probs")
                        nc.scalar.activation(probs[:], scps[:], AF.Exp)
                        nc.tensor.matmul(
                            sum_ps[th][:],
                            ones_sb[:],
                            probs[:],
                            start=(st == 0),
                            stop=(st == 7),
                        )
                        nc.tensor.matmul(
                            attn_ps[:, ts],
                            v_sb[:, st, :],
                            probs[:],
                            start=(st == 0),
                            stop=(st == 7),
                        )
                for th in range(TH):
                    ts = slice(th * FD, (th + 1) * FD)
                    recip = nrm_pool.tile([1, FD], f32r, tag="recip", name="recip")
                    nc.vector.reciprocal(recip[:], sum_ps[th][:])
                    bc_ps = bc_pool.tile([P, FD], f32, tag="bcps", name="bcps")
                    nc.tensor.matmul(
                        bc_ps[:], ones_row[0:1, :], recip[0:1, :], start=True, stop=True
                    )
                    bcast = nrm_pool.tile([P, FD], f32, tag="bcast", name="bcast")
                    nc.scalar.copy(bcast[:], bc_ps[:])
                    nc.vector.tensor_tensor(
                        attn_sb[h][:, ts], attn_ps[:, ts], bcast[:], ALU.mult
                    )

        # ---- phase 3: output projection (partial over this head group) ----
        with ExitStack() as p3:
            wo_pool = p3.enter_context(tc.tile_pool(name="wo", bufs=1))
            ou_pool = p3.enter_context(tc.tile_pool(name="ou", bufs=4))
            ops_pool = p3.enter_context(tc.tile_pool(name="ops", bufs=4, space="PSUM"))

            wo_sb = [wo_pool.tile([P, H], f32r, tag=f"wo{c}", name=f"wo{c}") for c in range(NHG)]
            for c in range(NHG):
                nc.sync.dma_start(wo_sb[c][:], woT[c * P : (c + 1) * P, :])

            for jt in range(H // P):
                for th in range(TH):
                    ts = slice(th * FD, (th + 1) * FD)
                    ops = ops_pool.tile([P, FD], f32, tag="ops", name="ops")
                    for c in range(NHG):
                        nc.tensor.matmul(
                            ops[:],
                            wo_sb[c][:, jt * P : (jt + 1) * P],
                            attn_sb[c][:, ts],
                            start=(c == 0),
                            stop=(c == NHG - 1),
                        )
                    o_sb = ou_pool.tile([P, FD], f32, tag="osb", name="osb")
                    nc.scalar.copy(o_sb[:], ops[:])
                    nc.sync.dma_start(outT[jt * P : (jt + 1) * P, ts], o_sb[:])

    _split_multiwaits(nc)
    return nc


_NC = None


def _get_nc():
    global _NC
    if _NC is None:
        _NC = _build_program()
    return _NC


def _make_in_maps(hidden_states, key, value, attention_mask, rope_cos, rope_sin, wq, bq, wo):
    scale = 1.0 / np.sqrt(HD)
    cosT = np.ascontiguousarray(rope_cos.T * scale).astype(np.float32)
    sinT = (rope_sin.T * scale).astype(np.float32)
    sinT = np.ascontiguousarray(np.concatenate([-sinT[: HD // 2], sinT[HD // 2 :]], axis=0))
    in_maps = []
    for c in range(8):
        b, g = c // 2, c % 2
        js = slice(g * JW, (g + 1) * JW)
        hs_b = np.ascontiguousarray(hidden_states[b].T)  # [H, S]
        wqT_c = np.ascontiguousarray(wq[js, :].T)  # [H, JW]
        bqT_c = np.ascontiguousarray(bq[js].reshape(NHG, P).T)  # [P, NHG]
        kk_c = np.ascontiguousarray(key[b * NH + g * NHG : b * NH + (g + 1) * NHG])
        vv_c = np.ascontiguousarray(value[b, g * NHG : (g + 1) * NHG])
        maskT_c = np.ascontiguousarray(np.broadcast_to(attention_mask[b, 0], (S, S)).T)
        woT_c = np.ascontiguousarray(wo[:, js].T)  # [JW, H]
        in_maps.append(
            {
                "hsT": hs_b,
                "wqT": wqT_c,
                "bqT": bqT_c,
                "kk": kk_c,
                "vv": vv_c,
                "maskT": maskT_c,
                "cosT": cosT,
                "sinT": sinT,
                "woT": woT_c,
                "ones_d": np.ones((P, P), dtype=np.float32),
            }
        )
    return in_maps


def _assemble(results, bo):
    out = np.empty((B, S, H), dtype=np.float32)
    for b in range(B):
        acc = results[2 * b]["outT"] + results[2 * b + 1]["outT"]  # [H, S]
        out[b] = acc.T + bo[None, :]
    return out


def kernel(hidden_states, key, value, attention_mask, rope_cos, rope_sin, wq, bq, wo, bo):
    nc = _get_nc()
    in_maps = _make_in_maps(
        hidden_states, key, value, attention_mask, rope_cos, rope_sin, wq, bq, wo
    )
    res = run_bass_kernel_spmd(nc, in_maps, list(range(8)))
    return _assemble(res.results, np.asarray(bo, dtype=np.float32))


def run_traced(hidden_states, key, value, attention_mask, rope_cos, rope_sin, wq, bq, wo, bo):
    """Like kernel() but with NTFF tracing; returns (output, BassKernelResults)."""
    nc = _get_nc()
    in_maps = _make_in_maps(
        hidden_states, key, value, attention_mask, rope_cos, rope_sin, wq, bq, wo
    )
    res = run_bass_kernel_spmd(nc, in_maps, list(range(8)), trace=True, trace_cores=[0])
    return _assemble(res.results, np.asarray(bo, dtype=np.float32)), res
